# revision 10
# baseline (speedup 1.0000x reference)
"""Trainium2 Bass kernel for multi-head self-attention (B=2, N=4096, C=512, H=8).

Sharding: 8 cores = 2 batches x 4 head-pairs. Core c handles batch c//4 and
heads {2*(c%4), 2*(c%4)+1}. Each core computes its two heads' attention over
all 4096 tokens and a partial output projection restricted to its heads' 128
channels; the host sums the 4 partials per batch (the tensor-parallel proj
all-reduce) and adds b_proj.

v2 dataflow (fp16 operands, fp32 PSUM accumulation, scores never in DRAM):
  xT arrives host-pre-transposed [512, 4096] fp16 -> SBUF (no PE transposes)
  kT/qT = (w^T @ xT)              [128, 4096]  (rows 0-63 head0, 64-127 head1)
  v natural = xT-block^T @ wv     [4096, 130]  per tile: [Vh0 | 1 | Vh1 | 1]
  per 512-query chunk, per 128-key tile:
    S^T = kslc^T @ qT   (two row-packed K=64 matmuls -> PSUM [128, 1024])
    P^T = exp(SCALE * S^T) fp16: column-split across engines every step --
          ScalarE exp on cols [0:XA], DVE Schraudolph exp2 on [XA:1024]
    PV accumulation uses P^T subtiles as the STATIONARY operand and [V|1] as
      the 65-col moving operand: 8 small matmuls (4 query-subtiles x 2 heads)
      -> per-query-subtile PSUM [128q, 65] accumulators; col 64 is the
      softmax denominator (free -- no ones-row drains, half the PV cycles).
  chunk epilogue (interleaved into the next chunk's key loop):
    batched reciprocal of the 8 denominator columns (DVE), 8 per-partition
    tensor_scalar normalizes (split ACT/DVE) -> fp16 [q, dd] tiles, 8 PE
    transposes -> fp16 PSUM outT^T, one 2x DVE copy -> SBUF, then ONE packed
    proj matmul per 128-query tile and fp16 partial DMA to DRAM.
"""

import os
import sys

if "/opt/trn_rl_repo" not in sys.path:
    sys.path.insert(0, "/opt/trn_rl_repo")

import numpy as np

import concourse.mybir as mybir
import concourse.tile as tile
from concourse import bacc

B, N, C, H = 2, 4096, 512, 8
D = C // H
SCALE = D**-0.5
F32 = mybir.dt.float32
F16 = mybir.dt.float16
I16 = mybir.dt.int16
MUL = mybir.AluOpType.mult
EXP = mybir.ActivationFunctionType.Exp
CPY = mybir.ActivationFunctionType.Copy

MM_DT_NAME = "f16"  # informational (test.py prints it)

# ScalarE handles cols [0:XA] of each 1024-col score tile; DVE the rest.
XA = int(os.environ.get("ATTN_ACT_COLS", "576"))
DEFER = int(os.environ.get("ATTN_DEFER", "6"))

_EXP_OP = None


def _get_exp_op():
    """Register (once) a custom DVE op computing fp16 exp bit patterns.

    Schraudolph in fp16: bits = relu(x*C0 + C1) with C0 = SCALE*log2e*1024,
    C1 = 15360 - 1024*sigma (sigma = 0.0579 makes the sawtooth mean-zero).
    Rounded to int16 at writeback, the result IS the fp16 encoding of
    ~e^(x*SCALE) (max rel err ~4%, zero-mean; the softmax ratio washes it
    out). relu clamps the x << 0 tail to +0.0.
    """
    global _EXP_OP
    if _EXP_OP is not None:
        return _EXP_OP
    from concourse import dve_ops
    from concourse.dve_spec import Spec, Src0, C0, C1, relu, lower
    from concourse.dve_uop import DveOpSpec

    name = "EXP2F16_ANT"
    spec = Spec(
        body=relu(Src0 * C0 + C1),
        # CoreSim reference (HW rounds the float result at int16 writeback)
        reference=lambda in0, in1, s0, s1, imm2: np.rint(
            np.maximum(in0.astype(np.float32) * s0 + s1, 0.0)
        ),
    )
    existing = [op for op in dve_ops.OPS if op.name == name]
    if existing:
        _EXP_OP = existing[0]
        return _EXP_OP
    opcode = dve_ops._CUSTOM_DVE_ROW_BASE + len(dve_ops.OPS)
    uops = lower(spec, ver="v3")
    ds = DveOpSpec(name=name, opcode=opcode, uops=uops, rd1_en=False)
    sha = ds.sha("v3")
    op = dve_ops.DveOp(name, spec, subdim=False, uops_sha={"v3": sha})
    dve_ops.OPS.append(op)
    dve_ops.CUSTOM_DVE_SPECS[name] = spec
    dve_ops._SUB_OPCODE_FOR_NAME[name] = opcode
    _EXP_OP = op
    return op


def _exp_consts():
    """(C0, C1) for the custom exp op."""
    import math

    c0 = SCALE * math.log2(math.e) * 1024.0
    c1 = 15360.0 - 1024.0 * 0.0579
    return c0, c1


def build(tokens=N):
    T = tokens
    n_xt = T // 128  # key tiles
    n_s = T // 512  # token slices for kT/qT production
    n_qc = T // 512  # query chunks

    exp_op = _get_exp_op()
    ec0, ec1 = _exp_consts()

    nc = bacc.Bacc(None)
    xt = nc.dram_tensor("xt", [C, T], F16, kind="ExternalInput")  # x[b].T
    out = nc.dram_tensor("out", [T, C], F16, kind="ExternalOutput")
    # concatenated [wq | wk | wv | wp | I128], each [128, 512]/[128,128] fp16
    # (wq/wk/wv: w_[p, kc*128 + j] = w[kc*128 + p, j]; wp natural rows)
    wall = nc.dram_tensor("wall", [128, 2176], F16, kind="ExternalInput")

    with tile.TileContext(nc) as tc:
        with tc.tile_pool(name="persist", bufs=1) as pp:
            w_all = pp.tile([128, 2176], F16, tag="w_all", name="w_all")
            nc.sync.dma_start(out=w_all[:], in_=wall[:, :])
            wq_sb = w_all[:, 0:512]
            wk_sb = w_all[:, 512:1024]
            wv_sb = w_all[:, 1024:1536]
            wp_sb = w_all[:, 1536:2048]
            ident = w_all[:, 2048:2176]
            # warm the Exp activation table (~1.3us) during the input DMAs
            # instead of on the first real exp
            dum = pp.tile([1, 1], F32, tag="dum")
            nc.gpsimd.memset(dum[:], 0.0)
            nc.scalar.activation(dum[:], dum[:], EXP, scale=1.0)

            # all 4 c-chunks of xT in one flat tile: chunk kc at cols [kc*T..)
            xall = pp.tile([128, 4 * T], F16, tag="xall", name="xall")

            def xslc(kc, sl):
                return xall[:, kc * T + sl.start : kc * T + sl.stop]

            kT = [
                pp.tile([128, 512], F16, tag=f"kT{s}", name=f"kT{s}")
                for s in range(n_s)
            ]
            qT = [
                pp.tile([128, 512], F16, tag=f"qT{s}", name=f"qT{s}")
                for s in range(n_s)
            ]
            v = [
                pp.tile([128, 130], F16, tag=f"v{t}", name=f"v{t}")
                for t in range(n_xt)
            ]

            def attn_S(qc, kt, psS, ptp):
                """Scores + column-split exp for one (chunk, key-tile)."""
                sc = psS.tile([128, 1024], F32, tag="sc", name="sc")
                kslc = kT[kt // 4][:, (kt % 4) * 128 : (kt % 4 + 1) * 128]
                nc.tensor.matmul(
                    sc[:, 0:512],
                    kslc[0:64, :],
                    qT[qc][0:64, :],
                    start=True,
                    stop=True,
                    tile_position=(0, 0),
                )
                nc.tensor.matmul(
                    sc[:, 512:1024],
                    kslc[64:128, :],
                    qT[qc][64:128, :],
                    start=True,
                    stop=True,
                    tile_position=(64, 0),
                )
                pt = ptp.tile([128, 1024], F16, tag="pt", name="pt")
                nc.scalar.activation(pt[:, 0:XA], sc[:, 0:XA], EXP, scale=SCALE)
                nc.vector._custom_dve(
                    exp_op,
                    out=pt[:, XA:1024].bitcast(I16),
                    in0=sc[:, XA:1024],
                    s0=ec0,
                    s1=ec1,
                )
                return pt

            def attn_PV(kt, pt, pv0, pv1, start=None, stop=None):
                """P^T-stationary PV: 8 K=128 matmuls with 65-col moving [V|1].

                pv_h[:, qs*65+64] accumulates the softmax denominator."""
                first = (kt == 0) if start is None else start
                last = (kt == n_xt - 1) if stop is None else stop
                # start=True zeroes the bank's whole 2KB zero-region, so only
                # the first matmul into each pv bank starts the group and only
                # the last one stops it; the 4 qs sub-regions share the group.
                for qs in range(4):
                    nc.tensor.matmul(
                        pv0[:, qs * 65 : qs * 65 + 65],
                        pt[:, qs * 128 : (qs + 1) * 128],
                        v[kt][:, 0:65],
                        start=first and qs == 0,
                        stop=last and qs == 3,
                    )
                    nc.tensor.matmul(
                        pv1[:, qs * 65 : qs * 65 + 65],
                        pt[:, 512 + qs * 128 : 512 + (qs + 1) * 128],
                        v[kt][:, 65:130],
                        start=first and qs == 0,
                        stop=last and qs == 3,
                    )

            def epi_normalize(pv0, pv1, smp):
                """Batched reciprocal of the 8 denominator columns, then 8
                per-partition-scalar normalizes (fp32 PSUM -> fp16 SBUF),
                split across ACT and DVE. Returns the [q, dd] tile."""
                rcp = smp.tile([128, 8], F32, tag="rcp", name="rcp")
                d0 = pv0[:, 0:260].rearrange("p (q f) -> p q f", f=65)[:, :, 64:65]
                d1 = pv1[:, 0:260].rearrange("p (q f) -> p q f", f=65)[:, :, 64:65]
                nc.vector.reciprocal(rcp[:, 0:4], d0)
                nc.vector.reciprocal(rcp[:, 4:8], d1)
                an = smp.tile([128, 512], F16, tag="an", name="an")
                for qs in range(4):
                    # head0 -> ACT (activation Copy with per-partition scale)
                    nc.scalar.activation(
                        an[:, qs * 128 : qs * 128 + 64],
                        pv0[:, qs * 65 : qs * 65 + 64],
                        CPY,
                        scale=rcp[:, qs : qs + 1],
                    )
                    # head1 -> DVE tensor_scalar mult
                    nc.vector.tensor_scalar(
                        an[:, qs * 128 + 64 : (qs + 1) * 128],
                        pv1[:, qs * 65 : qs * 65 + 64],
                        rcp[:, 4 + qs : 5 + qs],
                        None,
                        MUL,
                    )
                return an

            def epi_transpose(an, psT):
                """8 PE transposes: [128q, 64dd] fp16 -> outT^T fp16 PSUM."""
                pT = psT.tile([128, 512], F16, tag="pT", name="pT")
                for qs in range(4):
                    nc.tensor.transpose(
                        pT[0:64, qs * 128 : (qs + 1) * 128],
                        an[:, qs * 128 : qs * 128 + 64],
                        ident,
                    )
                    nc.tensor.transpose(
                        pT[64:128, qs * 128 : (qs + 1) * 128],
                        an[:, qs * 128 + 64 : (qs + 1) * 128],
                        ident,
                    )
                return pT

            def epi_outT(pT, otp):
                outT = otp.tile([128, 512], F16, tag="outT", name="outT")
                nc.vector.tensor_copy(outT[:], pT[:])
                return outT

            def proj_qtile(qc, qs, outT, psP, obp, on_act=False):
                i = qc * 4 + qs
                pj = psP.tile([128, 512], F32, tag="pj", name="pj")
                nc.tensor.matmul(
                    pj[:],
                    outT[:, qs * 128 : (qs + 1) * 128],
                    wp_sb[:],
                    start=True,
                    stop=True,
                )
                ob = obp.tile([128, 512], F16, tag="ob", name="ob")
                if on_act:
                    nc.scalar.activation(ob[:], pj[:], CPY)
                else:
                    nc.vector.tensor_copy(ob[:], pj[:])
                nc.sync.dma_start(out=out[i * 128 : (i + 1) * 128, :], in_=ob[:])

            with tc.tile_pool(name="ptp", bufs=10) as ptp, tc.tile_pool(
                name="smp", bufs=2
            ) as smp, tc.tile_pool(name="otp", bufs=2) as otp, tc.tile_pool(
                name="obp", bufs=3
            ) as obp, tc.tile_pool(
                name="psS", bufs=2, space="PSUM"
            ) as psS, tc.tile_pool(name="psV", bufs=1, space="PSUM") as psV:
                # per-head PV accumulators: 4 query-subtiles x [64d | den]
                # ([128, 512] so each accumulator owns a full PSUM bank; only
                # the first 260 cols are used)
                pv0 = psV.tile([128, 512], F32, tag="pv0", name="pv0")
                pv1 = psV.tile([128, 512], F32, tag="pv1", name="pv1")

                # ---- prologue: per 512-token slice produce kT/qT/v, with
                # qc=0's attention interleaved so ACT/DVE start early
                with tc.tile_pool(name="psA", bufs=2, space="PSUM") as psA:
                    for s in range(n_s):
                        sl = slice(s * 512, (s + 1) * 512)
                        # one DMA fetches this slice's tokens for all 4 chunks
                        nc.sync.dma_start(
                            out=xall[:].rearrange("p (k t) -> p k t", k=4)[:, :, sl],
                            in_=xt.rearrange("(k p) t -> p k t", k=4)[:, :, sl],
                        )
                        for w_sb, dst in ((wk_sb, kT), (wq_sb, qT)):
                            ps = psA.tile([128, 512], F32, tag="work", name="ps_kq")
                            for kc in range(4):
                                nc.tensor.matmul(
                                    ps[:],
                                    w_sb[:, kc * 128 : (kc + 1) * 128],
                                    xslc(kc, sl),
                                    start=(kc == 0),
                                    stop=(kc == 3),
                                )
                            nc.scalar.activation(dst[s][:], ps[:], CPY)
                        # S+exp for this slice's 4 key tiles first: both exp
                        # engines stream while the PE still produces v below.
                        pts = [
                            attn_S(0, kt, psS, ptp)
                            for kt in range(4 * s, 4 * s + 4)
                        ]
                        # v natural: per 128-token block, accumulate over kc
                        vn = psA.tile([128, 512], F32, tag="work", name="vn")
                        for j in range(4):
                            tb = slice((4 * s + j) * 128, (4 * s + j + 1) * 128)
                            for kc in range(4):
                                nc.tensor.matmul(
                                    vn[:, j * 128 : (j + 1) * 128],
                                    xslc(kc, tb),
                                    wv_sb[:, kc * 128 : (kc + 1) * 128],
                                    start=(kc == 0),
                                    stop=(kc == 3),
                                )
                        for j in range(4):
                            t = 4 * s + j
                            nc.vector.tensor_copy(
                                v[t][:, 0:64], vn[:, j * 128 : j * 128 + 64]
                            )
                            nc.vector.tensor_copy(
                                v[t][:, 65:129], vn[:, j * 128 + 64 : (j + 1) * 128]
                            )
                            nc.gpsimd.memset(v[t][:, 64:65], 1.0)
                            nc.gpsimd.memset(v[t][:, 129:130], 1.0)
                        for i, kt in enumerate(range(4 * s, 4 * s + 4)):
                            attn_PV(kt, pts[i], pv0, pv1)

                # ---- steady state: chunks 1..n_qc-1. Chunk qc-1's epilogue is
                # interleaved into chunk qc's kt loop: the first DEFER steps
                # emit only S+exp while the epilogue normalizes (and thereby
                # frees) the previous chunk's PV accumulators; their PV
                # matmuls are emitted after so the in-order PE never blocks.
                with tc.tile_pool(name="psX", bufs=1, space="PSUM") as psX, (
                    tc.tile_pool(name="psP", bufs=1, space="PSUM")
                ) as psP:
                    prev_qc = 0
                    for qc in range(1, n_qc):
                        pts = [attn_S(qc, kt, psS, ptp) for kt in range(DEFER)]
                        an = epi_normalize(pv0, pv1, smp)
                        pv0 = psV.tile([128, 512], F32, tag="pv0", name="pv0")
                        pv1 = psV.tile([128, 512], F32, tag="pv1", name="pv1")
                        for kt in range(DEFER):
                            attn_PV(kt, pts[kt], pv0, pv1)
                        outT = None
                        pT = None
                        pbase = DEFER + 2
                        pstep = max(1, min(3, (n_xt - pbase) // 4))
                        pj_kts = {pbase + qs * pstep: qs for qs in range(4)}
                        for kt in range(DEFER, n_xt):
                            pt = attn_S(qc, kt, psS, ptp)
                            attn_PV(kt, pt, pv0, pv1)
                            if kt == DEFER:
                                pT = epi_transpose(an, psX)
                            elif kt == DEFER + 1:
                                outT = epi_outT(pT, otp)
                            elif kt in pj_kts:
                                proj_qtile(prev_qc, pj_kts[kt], outT, psP, obp)
                        prev_qc = qc
                    # ---- tail epilogue for the last chunk (ACT is idle now)
                    an = epi_normalize(pv0, pv1, smp)
                    pT = epi_transpose(an, psX)
                    outT = epi_outT(pT, otp)
                    for qs in range(4):
                        proj_qtile(prev_qc, qs, outT, psP, obp, on_act=(qs % 2 == 0))
    nc.compile()
    return nc


_CACHE = {}


def _get_nc(tokens=N):
    if tokens not in _CACHE:
        _CACHE[tokens] = build(tokens)
    return _CACHE[tokens]


def _prep_w(w_slice):
    """[512, 128] -> [128, 512] fp16 with w_[p, kc*128 + j] = w[kc*128 + p, j]."""
    w = np.asarray(w_slice, dtype=np.float32)
    return np.ascontiguousarray(
        w.reshape(4, 128, 128).transpose(1, 0, 2).reshape(128, 512).astype(np.float16)
    )


def _shard_inputs(x, w_qkv, w_proj):
    ident = np.eye(128, dtype=np.float16)
    in_maps = []
    for c in range(8):
        b, hp = divmod(c, 4)
        o = 128 * hp
        wall = np.concatenate(
            [
                _prep_w(w_qkv[:, o : o + 128]),
                _prep_w(w_qkv[:, 512 + o : 512 + o + 128]),
                _prep_w(w_qkv[:, 1024 + o : 1024 + o + 128]),
                w_proj[o : o + 128, :].astype(np.float16),
                ident,
            ],
            axis=1,
        )
        in_maps.append(
            {
                "xt": np.ascontiguousarray(x[b].T.astype(np.float16)),
                "wall": np.ascontiguousarray(wall),
            }
        )
    return in_maps


def run(x, w_qkv, w_proj, b_proj, trace=False, **kwargs):
    from concourse.bass_utils import run_bass_kernel_spmd

    nc = _get_nc()
    in_maps = _shard_inputs(np.asarray(x), np.asarray(w_qkv), np.asarray(w_proj))
    br = run_bass_kernel_spmd(nc, in_maps, list(range(8)), trace=trace, **kwargs)
    parts = [
        np.asarray(br.results[c]["out"]).astype(np.float32) for c in range(8)
    ]
    bp = np.asarray(b_proj)
    o0 = parts[0] + parts[1] + parts[2] + parts[3] + bp
    o1 = parts[4] + parts[5] + parts[6] + parts[7] + bp
    return np.stack([o0, o1]).astype(np.float32), br


def kernel(x, w_qkv, w_proj, b_proj):
    result, _ = run(x, w_qkv, w_proj, b_proj, trace=False)
    return result


# revision 18
# speedup vs baseline: 1.1465x; 1.1465x over previous
"""Trainium2 Bass kernel for multi-head self-attention (B=2, N=4096, C=512, H=8).

Sharding: 8 cores = 2 batches x 4 head-pairs. Core c handles batch c//4 and
heads {2*(c%4), 2*(c%4)+1}. Each core computes its two heads' attention over
all 4096 tokens and a partial output projection restricted to its heads' 128
channels; the host sums the 4 partials per batch (the tensor-parallel proj
all-reduce) and adds b_proj.

Dataflow (fp16 operands, fp32 PSUM accumulation, scores never in DRAM):
  xT arrives host-pre-transposed [512, 4096] fp16 -> SBUF (no PE transposes)
  kT/qT = (w^T @ xT)              [128, 4096]  (rows 0-63 head0, 64-127 head1)
  v natural = xT-block^T @ wv     [4096, 130]  per tile: [Vh0 | 1 | Vh1 | 1]
  per 512-query chunk, per 128-key tile:
    S^T = kslc^T @ qT   (two row-packed K=64 matmuls -> PSUM [128, 1024])
    P^T = exp(SCALE * S^T) fp16   (ScalarE, straight out of PSUM; a fraction
                                   of steps use a custom DVE exp2 instead to
                                   share the exp load with the vector engine)
    PV += [V|1]^T @ P^T           (PSUM accumulate; row 64 = denominators)
  chunk epilogue: raw PV drain (frees PSUM fast, hidden under deferred S/exp
    of the next chunk), then recip/broadcast/scale off the critical path and
    ONE packed proj matmul per 128-query tile (both heads in one K=128).
"""

import os
import sys

if "/opt/trn_rl_repo" not in sys.path:
    sys.path.insert(0, "/opt/trn_rl_repo")

import numpy as np

import concourse.mybir as mybir
import concourse.tile as tile
from concourse import bacc

B, N, C, H = 2, 4096, 512, 8
D = C // H
SCALE = D**-0.5
F32 = mybir.dt.float32
F16 = mybir.dt.float16
I16 = mybir.dt.int16
MUL = mybir.AluOpType.mult
EXP = mybir.ActivationFunctionType.Exp

MM_DT_NAME = "f16"  # informational (test.py prints it)

# Every step's exp is column-split across engines: ScalarE Exp on cols
# [0:XA], DVE Schraudolph exp2 (custom op, int16 writeback) on [XA:1024].
XA = int(os.environ.get("ATTN_ACT_COLS", "640"))

_EXP_OP = None


def _get_exp_op():
    """Register (once) a custom DVE op computing fp16 exp bit patterns.

    Schraudolph in fp16: bits = relu(x*C0 + C1) with C0 = SCALE*log2e*1024,
    C1 = 15360 - 1024*sigma (sigma = 0.0579 makes the sawtooth mean-zero).
    Rounded to int16 at writeback, the result IS the fp16 encoding of
    ~e^(x*SCALE) (max rel err ~4%, zero-mean; the softmax ratio washes it
    out). relu clamps the x << 0 tail to +0.0.
    """
    global _EXP_OP
    if _EXP_OP is not None:
        return _EXP_OP
    from concourse import dve_ops
    from concourse.dve_spec import Spec, Src0, C0, C1, relu, lower
    from concourse.dve_uop import DveOpSpec

    name = "EXP2F16_ANT"
    spec = Spec(
        body=relu(Src0 * C0 + C1),
        # CoreSim reference (HW rounds the float result at int16 writeback)
        reference=lambda in0, in1, s0, s1, imm2: np.rint(
            np.maximum(in0.astype(np.float32) * s0 + s1, 0.0)
        ),
    )
    existing = [op for op in dve_ops.OPS if op.name == name]
    if existing:
        _EXP_OP = existing[0]
        return _EXP_OP
    opcode = dve_ops._CUSTOM_DVE_ROW_BASE + len(dve_ops.OPS)
    uops = lower(spec, ver="v3")
    ds = DveOpSpec(name=name, opcode=opcode, uops=uops, rd1_en=False)
    sha = ds.sha("v3")
    op = dve_ops.DveOp(name, spec, subdim=False, uops_sha={"v3": sha})
    dve_ops.OPS.append(op)
    dve_ops.CUSTOM_DVE_SPECS[name] = spec
    dve_ops._SUB_OPCODE_FOR_NAME[name] = opcode
    _EXP_OP = op
    return op


def _exp_consts():
    """(C0, C1) for the custom exp op."""
    import math

    c0 = SCALE * math.log2(math.e) * 1024.0
    c1 = 15360.0 - 1024.0 * 0.0579
    return c0, c1


def build(tokens=N):
    T = tokens
    n_xt = T // 128  # key tiles
    n_s = T // 512  # token slices for kT/qT production
    n_qc = T // 512  # query chunks

    exp_op = _get_exp_op()
    ec0, ec1 = _exp_consts()

    nc = bacc.Bacc(None)
    xt = nc.dram_tensor("xt", [C, T], F16, kind="ExternalInput")  # x[b].T
    out = nc.dram_tensor("out", [T, C], F32, kind="ExternalOutput")
    # concatenated [wq | wk | wv | wp], each [128, 512] host-prepped fp16
    # (wq/wk/wv: w_[p, kc*128 + j] = w[kc*128 + p, j]; wp natural rows)
    wall = nc.dram_tensor("wall", [128, 2048], F16, kind="ExternalInput")

    with tile.TileContext(nc) as tc:
        with tc.tile_pool(name="persist", bufs=1) as pp:
            w_all = pp.tile([128, 2048], F16, tag="w_all", name="w_all")
            nc.sync.dma_start(out=w_all[:], in_=wall[:, :])
            wq_sb = w_all[:, 0:512]
            wk_sb = w_all[:, 512:1024]
            wv_sb = w_all[:, 1024:1536]
            wp_sb = w_all[:, 1536:2048]
            # ones row for broadcasting recip rows across 64 partitions
            ones1 = pp.tile([1, 64], F16, tag="ones1")
            nc.gpsimd.memset(ones1[:], 1.0)
            # warm the Exp activation table (~1.3us) during the input DMAs
            # instead of on the first real exp
            dum = pp.tile([1, 1], F32, tag="dum")
            nc.gpsimd.memset(dum[:], 0.0)
            nc.scalar.activation(dum[:], dum[:], EXP, scale=1.0)

            # all 4 c-chunks of xT in one flat tile: chunk kc at cols [kc*T..)
            xall = pp.tile([128, 4 * T], F16, tag="xall", name="xall")

            def xslc(kc, sl):
                return xall[:, kc * T + sl.start : kc * T + sl.stop]
            kT = [
                pp.tile([128, 512], F16, tag=f"kT{s}", name=f"kT{s}")
                for s in range(n_s)
            ]
            qT = [
                pp.tile([128, 512], F16, tag=f"qT{s}", name=f"qT{s}")
                for s in range(n_s)
            ]
            v = [
                pp.tile([128, 130], F16, tag=f"v{t}", name=f"v{t}")
                for t in range(n_xt)
            ]

            def attn_S(qc, kt, psS, ptp):
                """Scores + column-split exp for one (chunk, key-tile)."""
                sc = psS.tile([128, 1024], F32, tag="sc", name="sc")
                kslc = kT[kt // 4][:, (kt % 4) * 128 : (kt % 4 + 1) * 128]
                nc.tensor.matmul(
                    sc[:, 0:512],
                    kslc[0:64, :],
                    qT[qc][0:64, :],
                    start=True,
                    stop=True,
                    tile_position=(0, 0),
                )
                nc.tensor.matmul(
                    sc[:, 512:1024],
                    kslc[64:128, :],
                    qT[qc][64:128, :],
                    start=True,
                    stop=True,
                    tile_position=(64, 0),
                )
                pt = ptp.tile([128, 1024], F16, tag="pt", name="pt")
                nc.scalar.activation(pt[:, 0:XA], sc[:, 0:XA], EXP, scale=SCALE)
                nc.vector._custom_dve(
                    exp_op,
                    out=pt[:, XA:1024].bitcast(I16),
                    in0=sc[:, XA:1024],
                    s0=ec0,
                    s1=ec1,
                )
                return pt

            def attn_PV(kt, pt, pv0, pv1, start=None, stop=None):
                start = (kt == 0) if start is None else start
                stop = (kt == n_xt - 1) if stop is None else stop
                nc.tensor.matmul(
                    pv0[:], v[kt][:, 0:65], pt[:, 0:512], start=start, stop=stop
                )
                nc.tensor.matmul(
                    pv1[:], v[kt][:, 65:130], pt[:, 512:1024], start=start, stop=stop
                )

            CPY = mybir.ActivationFunctionType.Copy

            def epi_drain(pv0, pv1, smp, otp, final=False):
                """Fast PV-bank release: raw copies only (DVE, ~2.7us). In the
                final epilogue ScalarE is idle (no more exps) and `copy` lives
                in the exp activation table, so the praw halves go there to
                halve the serial drain chain."""
                dna = smp.tile([1, 512], F32, tag="dna", name="dna")
                dnb = smp.tile([1, 512], F32, tag="dnb", name="dnb")
                nc.vector.tensor_copy(dna[:], pv0[64:65, :])
                nc.vector.tensor_copy(dnb[:], pv1[64:65, :])
                praw = otp.tile([128, 512], F32, tag="praw", name="praw")
                # one half on ACT, one on DVE (both on ACT in the final
                # epilogue, when no more exps are coming)
                nc.scalar.activation(praw[0:64, :], pv0[0:64, :], CPY)
                if final:
                    nc.scalar.activation(praw[64:128, :], pv1[0:64, :], CPY)
                else:
                    nc.vector.tensor_copy(praw[64:128, :], pv1[0:64, :])
                return dna, dnb, praw

            def epi_scale(dna, dnb, praw, smp, otp, psB):
                """Off-critical-path: recip, broadcast matmuls, fused scale."""
                rca = smp.tile([1, 512], F32, tag="rca", name="rca")
                rcb = smp.tile([1, 512], F32, tag="rcb", name="rcb")
                nc.vector.reciprocal_approx_fast(rca[:], dna[:])
                nc.vector.reciprocal_approx_fast(rcb[:], dnb[:])
                rha = smp.tile([1, 512], F16, tag="rha", name="rha")
                rhb = smp.tile([1, 512], F16, tag="rhb", name="rhb")
                nc.vector.tensor_copy(rha[:], rca[:])
                nc.vector.tensor_copy(rhb[:], rcb[:])
                bc = psB.tile([128, 512], F32, tag="pb", name="bc")
                nc.tensor.matmul(bc[0:64, :], ones1[:], rha[:], start=True, stop=True)
                nc.tensor.matmul(
                    bc[64:128, :],
                    ones1[:],
                    rhb[:],
                    start=True,
                    stop=True,
                    tile_position=(0, 64),
                )
                outT = otp.tile([128, 512], F16, tag="outT", name="outT")
                nc.vector.tensor_tensor(outT[:], praw[:], bc[:], MUL)
                return outT

            def proj_qtile(qc, qs, outT, psP, obp, final=False):
                i = qc * 4 + qs
                pj = psP.tile([128, 512], F32, tag="pb", name="pj")
                nc.tensor.matmul(
                    pj[:],
                    outT[:, qs * 128 : (qs + 1) * 128],
                    wp_sb[:],
                    start=True,
                    stop=True,
                )
                ob = obp.tile([128, 512], F32, tag="ob", name="ob")
                if qs % 2 == 1:
                    nc.scalar.activation(ob[:], pj[:], CPY)
                else:
                    nc.vector.tensor_copy(ob[:], pj[:])
                nc.sync.dma_start(out=out[i * 128 : (i + 1) * 128, :], in_=ob[:])

            with tc.tile_pool(name="ptp", bufs=10) as ptp, tc.tile_pool(
                name="smp", bufs=2
            ) as smp, tc.tile_pool(name="otp", bufs=2) as otp, tc.tile_pool(
                name="obp", bufs=2
            ) as obp, tc.tile_pool(
                name="psS", bufs=2, space="PSUM"
            ) as psS, tc.tile_pool(name="psV", bufs=1, space="PSUM") as psV:
                pv0 = psV.tile([65, 512], F32, tag="pv0", name="pv0")
                pv1 = psV.tile([65, 512], F32, tag="pv1", name="pv1")
                prebaked = {}

                # ---- prologue: per 512-token slice produce kT/qT/v, with
                # qc=0's attention interleaved so ScalarE starts early
                with tc.tile_pool(name="psA", bufs=2, space="PSUM") as psA:
                    for s in range(n_s):
                        sl = slice(s * 512, (s + 1) * 512)
                        # one DMA fetches this slice's tokens for all 4 chunks
                        nc.sync.dma_start(
                            out=xall[:].rearrange("p (k t) -> p k t", k=4)[:, :, sl],
                            in_=xt.rearrange("(k p) t -> p k t", k=4)[:, :, sl],
                        )
                        for w_sb, dst in ((wk_sb, kT), (wq_sb, qT)):
                            ps = psA.tile([128, 512], F32, tag="work", name="ps_kq")
                            for kc in range(4):
                                nc.tensor.matmul(
                                    ps[:],
                                    w_sb[:, kc * 128 : (kc + 1) * 128],
                                    xslc(kc, sl),
                                    start=(kc == 0),
                                    stop=(kc == 3),
                                )
                            # ScalarE drains kT/qT: keeps the DVE queue clear
                            # for the v copies and uses Act's prologue slack
                            nc.scalar.activation(dst[s][:], ps[:], CPY)
                        # S+exp for this slice's 4 key tiles first: ScalarE
                        # streams exps while the PE still produces v below.
                        pts = [
                            attn_S(0, kt, psS, ptp)
                            for kt in range(4 * s, 4 * s + 4)
                        ]
                        # v natural: per 128-token block, accumulate over kc
                        vn = psA.tile([128, 512], F32, tag="work", name="vn")
                        for j in range(4):
                            tb = slice((4 * s + j) * 128, (4 * s + j + 1) * 128)
                            for kc in range(4):
                                nc.tensor.matmul(
                                    vn[:, j * 128 : (j + 1) * 128],
                                    xslc(kc, tb),
                                    wv_sb[:, kc * 128 : (kc + 1) * 128],
                                    start=(kc == 0),
                                    stop=(kc == 3),
                                )
                        for j in range(4):
                            t = 4 * s + j
                            nc.vector.tensor_copy(
                                v[t][:, 0:64], vn[:, j * 128 : j * 128 + 64]
                            )
                            nc.vector.tensor_copy(
                                v[t][:, 65:129], vn[:, j * 128 + 64 : (j + 1) * 128]
                            )
                            nc.gpsimd.memset(v[t][:, 64:65], 1.0)
                            nc.gpsimd.memset(v[t][:, 129:130], 1.0)
                        for i, kt in enumerate(range(4 * s, 4 * s + 4)):
                            attn_PV(kt, pts[i], pv0, pv1)
                        # pre-bake chunk 1's first steps into prologue slack:
                        # ScalarE picks up extra exps where it would idle; the
                        # parked pt tiles feed chunk 1's deferred-PV phase.
                        if s in (2, 3) and n_qc > 1:
                            for kt in (2 * (s - 2), 2 * (s - 2) + 1):
                                prebaked[(1, kt)] = attn_S(1, kt, psS, ptp)

                # ---- steady state: chunks 1..n_qc-1. Chunk qc-1's epilogue is
                # interleaved into chunk qc's kt loop: the first DEFER steps
                # emit only S+exp while the DVE drains the previous chunk's PV
                # accumulators; their PV matmuls are emitted after the drain so
                # the in-order PE never blocks.
                DEFER = 8
                # bc + pj share one 2-buffer pool (same tag): 2 PSUM banks
                # total, and consecutive projections ping-pong between banks.
                with tc.tile_pool(name="psX", bufs=2, space="PSUM") as psX:
                    psB = psP = psX
                    prev_pv = (pv0, pv1)
                    prev_qc = 0
                    outT = None
                    for qc in range(1, n_qc):
                        pv0 = psV.tile([65, 512], F32, tag="pv0", name="pv0")
                        pv1 = psV.tile([65, 512], F32, tag="pv1", name="pv1")
                        pts = [
                            prebaked.pop((qc, kt), None)
                            or attn_S(qc, kt, psS, ptp)
                            for kt in range(DEFER)
                        ]
                        drained = epi_drain(prev_pv[0], prev_pv[1], smp, otp)
                        for kt in range(DEFER):
                            attn_PV(kt, pts[kt], pv0, pv1)
                        for kt in range(DEFER, n_xt):
                            pt = attn_S(qc, kt, psS, ptp)
                            attn_PV(kt, pt, pv0, pv1)
                            if kt == min(DEFER + 1, 9):
                                outT = epi_scale(*drained, smp, otp, psB)
                            if kt in (9, 12, 15, 18):
                                proj_qtile(prev_qc, (kt - 9) // 3, outT, psP, obp)
                        prev_pv = (pv0, pv1)
                        prev_qc = qc
                    drained = epi_drain(prev_pv[0], prev_pv[1], smp, otp, final=True)
                    outT = epi_scale(*drained, smp, otp, psB)
                    for qs in range(4):
                        proj_qtile(prev_qc, qs, outT, psP, obp, final=True)
    nc.compile()
    return nc


_CACHE = {}


def _get_nc(tokens=N):
    if tokens not in _CACHE:
        _CACHE[tokens] = build(tokens)
    return _CACHE[tokens]


def _prep_w(w_slice):
    """[512, 128] -> [128, 512] fp16 with w_[p, kc*128 + j] = w[kc*128 + p, j]."""
    w = np.asarray(w_slice, dtype=np.float32)
    return np.ascontiguousarray(
        w.reshape(4, 128, 128).transpose(1, 0, 2).reshape(128, 512).astype(np.float16)
    )


def _shard_inputs(x, w_qkv, w_proj):
    in_maps = []
    for c in range(8):
        b, hp = divmod(c, 4)
        o = 128 * hp
        wall = np.concatenate(
            [
                _prep_w(w_qkv[:, o : o + 128]),
                _prep_w(w_qkv[:, 512 + o : 512 + o + 128]),
                _prep_w(w_qkv[:, 1024 + o : 1024 + o + 128]),
                w_proj[o : o + 128, :].astype(np.float16),
            ],
            axis=1,
        )
        in_maps.append(
            {
                "xt": np.ascontiguousarray(x[b].T.astype(np.float16)),
                "wall": np.ascontiguousarray(wall),
            }
        )
    return in_maps


def run(x, w_qkv, w_proj, b_proj, trace=False, **kwargs):
    from concourse.bass_utils import run_bass_kernel_spmd

    nc = _get_nc()
    in_maps = _shard_inputs(np.asarray(x), np.asarray(w_qkv), np.asarray(w_proj))
    br = run_bass_kernel_spmd(nc, in_maps, list(range(8)), trace=trace, **kwargs)
    parts = [np.asarray(br.results[c]["out"]) for c in range(8)]
    bp = np.asarray(b_proj)
    o0 = parts[0] + parts[1] + parts[2] + parts[3] + bp
    o1 = parts[4] + parts[5] + parts[6] + parts[7] + bp
    return np.stack([o0, o1]).astype(np.float32), br


def kernel(x, w_qkv, w_proj, b_proj):
    result, _ = run(x, w_qkv, w_proj, b_proj, trace=False)
    return result



# revision 22
# speedup vs baseline: 1.1482x; 1.0015x over previous
"""Trainium2 Bass kernel for multi-head self-attention (B=2, N=4096, C=512, H=8).

Sharding: 8 cores = 2 batches x 4 head-pairs. Core c handles batch c//4 and
heads {2*(c%4), 2*(c%4)+1}. Each core computes its two heads' attention over
all 4096 tokens and a partial output projection restricted to its heads' 128
channels; the host sums the 4 partials per batch (the tensor-parallel proj
all-reduce) and adds b_proj.

Dataflow (fp16 operands, fp32 PSUM accumulation, scores never in DRAM):
  xT arrives host-pre-transposed [512, 4096] fp16 -> SBUF (no PE transposes)
  kT/qT = (w^T @ xT)              [128, 4096]  (rows 0-63 head0, 64-127 head1)
  v natural = xT-block^T @ wv     [4096, 130]  per tile: [Vh0 | 1 | Vh1 | 1]
  per 512-query chunk, per 128-key tile:
    S^T = kslc^T @ qT   (two row-packed K=64 matmuls -> PSUM [128, 1024])
    P^T = exp(SCALE * S^T) fp16   (ScalarE, straight out of PSUM; a fraction
                                   of steps use a custom DVE exp2 instead to
                                   share the exp load with the vector engine)
    PV += [V|1]^T @ P^T           (PSUM accumulate; row 64 = denominators)
  chunk epilogue: raw PV drain (frees PSUM fast, hidden under deferred S/exp
    of the next chunk), then recip/broadcast/scale off the critical path and
    ONE packed proj matmul per 128-query tile (both heads in one K=128).
"""

import os
import sys

if "/opt/trn_rl_repo" not in sys.path:
    sys.path.insert(0, "/opt/trn_rl_repo")

import numpy as np

import concourse.mybir as mybir
import concourse.tile as tile
from concourse import bacc

B, N, C, H = 2, 4096, 512, 8
D = C // H
SCALE = D**-0.5
F32 = mybir.dt.float32
F16 = mybir.dt.float16
I16 = mybir.dt.int16
MUL = mybir.AluOpType.mult
EXP = mybir.ActivationFunctionType.Exp

MM_DT_NAME = "f16"  # informational (test.py prints it)

# Every step's exp is head-split across engines: ScalarE Exp computes head0's
# P^T tile, the DVE computes head1's via a custom Schraudolph exp2 op (int16
# writeback). Separate destination tiles keep the writers independent.

_EXP_OP = None


def _get_exp_op():
    """Register (once) a custom DVE op computing fp16 exp bit patterns.

    Schraudolph in fp16: bits = relu(x*C0 + C1) with C0 = SCALE*log2e*1024,
    C1 = 15360 - 1024*sigma (sigma = 0.0579 makes the sawtooth mean-zero).
    Rounded to int16 at writeback, the result IS the fp16 encoding of
    ~e^(x*SCALE) (max rel err ~4%, zero-mean; the softmax ratio washes it
    out). relu clamps the x << 0 tail to +0.0.
    """
    global _EXP_OP
    if _EXP_OP is not None:
        return _EXP_OP
    from concourse import dve_ops
    from concourse.dve_spec import Spec, Src0, C0, C1, relu, lower
    from concourse.dve_uop import DveOpSpec

    name = "EXP2F16_ANT"
    spec = Spec(
        body=relu(Src0 * C0 + C1),
        # CoreSim reference (HW rounds the float result at int16 writeback)
        reference=lambda in0, in1, s0, s1, imm2: np.rint(
            np.maximum(in0.astype(np.float32) * s0 + s1, 0.0)
        ),
    )
    existing = [op for op in dve_ops.OPS if op.name == name]
    if existing:
        _EXP_OP = existing[0]
        return _EXP_OP
    opcode = dve_ops._CUSTOM_DVE_ROW_BASE + len(dve_ops.OPS)
    uops = lower(spec, ver="v3")
    ds = DveOpSpec(name=name, opcode=opcode, uops=uops, rd1_en=False)
    sha = ds.sha("v3")
    op = dve_ops.DveOp(name, spec, subdim=False, uops_sha={"v3": sha})
    dve_ops.OPS.append(op)
    dve_ops.CUSTOM_DVE_SPECS[name] = spec
    dve_ops._SUB_OPCODE_FOR_NAME[name] = opcode
    _EXP_OP = op
    return op


def _exp_consts():
    """(C0, C1) for the custom exp op."""
    import math

    c0 = SCALE * math.log2(math.e) * 1024.0
    c1 = 15360.0 - 1024.0 * 0.0579
    return c0, c1


def build(tokens=N):
    T = tokens
    n_xt = T // 128  # key tiles
    n_s = T // 512  # token slices for kT/qT production
    n_qc = T // 512  # query chunks

    exp_op = _get_exp_op()
    ec0, ec1 = _exp_consts()

    nc = bacc.Bacc(None)
    xt = nc.dram_tensor("xt", [C, T], F16, kind="ExternalInput")  # x[b].T
    out = nc.dram_tensor("out", [T, C], F32, kind="ExternalOutput")
    # concatenated [wq | wk | wv | wp], each [128, 512] host-prepped fp16
    # (wq/wk/wv: w_[p, kc*128 + j] = w[kc*128 + p, j]; wp natural rows)
    wall = nc.dram_tensor("wall", [128, 2048], F16, kind="ExternalInput")

    with tile.TileContext(nc) as tc:
        with tc.tile_pool(name="persist", bufs=1) as pp:
            w_all = pp.tile([128, 2048], F16, tag="w_all", name="w_all")
            nc.sync.dma_start(out=w_all[:], in_=wall[:, :])
            wq_sb = w_all[:, 0:512]
            wk_sb = w_all[:, 512:1024]
            wv_sb = w_all[:, 1024:1536]
            wp_sb = w_all[:, 1536:2048]
            # ones row for broadcasting recip rows across 64 partitions
            ones1 = pp.tile([1, 64], F16, tag="ones1")
            nc.gpsimd.memset(ones1[:], 1.0)
            # warm the Exp activation table (~1.3us) during the input DMAs
            # instead of on the first real exp
            dum = pp.tile([1, 1], F32, tag="dum")
            nc.gpsimd.memset(dum[:], 0.0)
            nc.scalar.activation(dum[:], dum[:], EXP, scale=1.0)

            # all 4 c-chunks of xT in one flat tile: chunk kc at cols [kc*T..)
            xall = pp.tile([128, 4 * T], F16, tag="xall", name="xall")

            def xslc(kc, sl):
                return xall[:, kc * T + sl.start : kc * T + sl.stop]
            kT = [
                pp.tile([128, 512], F16, tag=f"kT{s}", name=f"kT{s}")
                for s in range(n_s)
            ]
            qT = [
                pp.tile([128, 512], F16, tag=f"qT{s}", name=f"qT{s}")
                for s in range(n_s)
            ]
            v = [
                pp.tile([128, 130], F16, tag=f"v{t}", name=f"v{t}")
                for t in range(n_xt)
            ]

            def attn_S(qc, kt, psS, ptp):
                """Scores + column-split exp for one (chunk, key-tile)."""
                sc = psS.tile([128, 1024], F32, tag="sc", name="sc")
                kslc = kT[kt // 4][:, (kt % 4) * 128 : (kt % 4 + 1) * 128]
                nc.tensor.matmul(
                    sc[:, 0:512],
                    kslc[0:64, :],
                    qT[qc][0:64, :],
                    start=True,
                    stop=True,
                    tile_position=(0, 0),
                )
                nc.tensor.matmul(
                    sc[:, 512:1024],
                    kslc[64:128, :],
                    qT[qc][64:128, :],
                    start=True,
                    stop=True,
                    tile_position=(64, 0),
                )
                pta = ptp.tile([128, 512], F16, tag="pta", name="pta")
                ptb = ptp.tile([128, 512], F16, tag="ptb", name="ptb")
                nc.scalar.activation(pta[:], sc[:, 0:512], EXP, scale=SCALE)
                nc.vector._custom_dve(
                    exp_op,
                    out=ptb[:].bitcast(I16),
                    in0=sc[:, 512:1024],
                    s0=ec0,
                    s1=ec1,
                )
                return pta, ptb

            def attn_PV(kt, pt, pv0, pv1, start=None, stop=None):
                pta, ptb = pt
                start = (kt == 0) if start is None else start
                stop = (kt == n_xt - 1) if stop is None else stop
                nc.tensor.matmul(
                    pv0[:], v[kt][:, 0:65], pta[:], start=start, stop=stop
                )
                nc.tensor.matmul(
                    pv1[:], v[kt][:, 65:130], ptb[:], start=start, stop=stop
                )

            CPY = mybir.ActivationFunctionType.Copy

            def epi_drain(pv0, pv1, smp, otp, final=False):
                """Fast PV-bank release: raw copies only (DVE, ~2.7us). In the
                final epilogue ScalarE is idle (no more exps) and `copy` lives
                in the exp activation table, so the praw halves go there to
                halve the serial drain chain."""
                dna = smp.tile([1, 512], F32, tag="dna", name="dna")
                dnb = smp.tile([1, 512], F32, tag="dnb", name="dnb")
                nc.vector.tensor_copy(dna[:], pv0[64:65, :])
                nc.vector.tensor_copy(dnb[:], pv1[64:65, :])
                praw = otp.tile([128, 512], F32, tag="praw", name="praw")
                # one half on ACT, one on DVE (both on ACT in the final
                # epilogue, when no more exps are coming)
                nc.scalar.activation(praw[0:64, :], pv0[0:64, :], CPY)
                if final:
                    nc.scalar.activation(praw[64:128, :], pv1[0:64, :], CPY)
                else:
                    nc.vector.tensor_copy(praw[64:128, :], pv1[0:64, :])
                return dna, dnb, praw

            def epi_scale(dna, dnb, praw, smp, otp, psB):
                """Off-critical-path: recip, broadcast matmuls, fused scale."""
                rca = smp.tile([1, 512], F32, tag="rca", name="rca")
                rcb = smp.tile([1, 512], F32, tag="rcb", name="rcb")
                nc.vector.reciprocal_approx_fast(rca[:], dna[:])
                nc.vector.reciprocal_approx_fast(rcb[:], dnb[:])
                rha = smp.tile([1, 512], F16, tag="rha", name="rha")
                rhb = smp.tile([1, 512], F16, tag="rhb", name="rhb")
                nc.vector.tensor_copy(rha[:], rca[:])
                nc.vector.tensor_copy(rhb[:], rcb[:])
                bc = psB.tile([128, 512], F32, tag="pb", name="bc")
                nc.tensor.matmul(bc[0:64, :], ones1[:], rha[:], start=True, stop=True)
                nc.tensor.matmul(
                    bc[64:128, :],
                    ones1[:],
                    rhb[:],
                    start=True,
                    stop=True,
                    tile_position=(0, 64),
                )
                outT = otp.tile([128, 512], F16, tag="outT", name="outT")
                nc.vector.tensor_tensor(outT[:], praw[:], bc[:], MUL)
                return outT

            def proj_qtile(qc, qs, outT, psP, obp, final=False):
                i = qc * 4 + qs
                pj = psP.tile([128, 512], F32, tag="pb", name="pj")
                nc.tensor.matmul(
                    pj[:],
                    outT[:, qs * 128 : (qs + 1) * 128],
                    wp_sb[:],
                    start=True,
                    stop=True,
                )
                ob = obp.tile([128, 512], F32, tag="ob", name="ob")
                if qs % 2 == 1:
                    nc.scalar.activation(ob[:], pj[:], CPY)
                else:
                    nc.vector.tensor_copy(ob[:], pj[:])
                nc.sync.dma_start(out=out[i * 128 : (i + 1) * 128, :], in_=ob[:])

            with tc.tile_pool(name="ptp", bufs=10) as ptp, tc.tile_pool(
                name="smp", bufs=2
            ) as smp, tc.tile_pool(name="otp", bufs=2) as otp, tc.tile_pool(
                name="obp", bufs=2
            ) as obp, tc.tile_pool(
                name="psS", bufs=2, space="PSUM"
            ) as psS, tc.tile_pool(name="psV", bufs=1, space="PSUM") as psV:
                pv0 = psV.tile([65, 512], F32, tag="pv0", name="pv0")
                pv1 = psV.tile([65, 512], F32, tag="pv1", name="pv1")
                prebaked = {}

                # ---- prologue: per 512-token slice produce kT/qT/v, with
                # qc=0's attention interleaved so ScalarE starts early
                with tc.tile_pool(name="psA", bufs=2, space="PSUM") as psA:
                    for s in range(n_s):
                        sl = slice(s * 512, (s + 1) * 512)
                        # one DMA fetches this slice's tokens for all 4 chunks
                        nc.sync.dma_start(
                            out=xall[:].rearrange("p (k t) -> p k t", k=4)[:, :, sl],
                            in_=xt.rearrange("(k p) t -> p k t", k=4)[:, :, sl],
                        )
                        for w_sb, dst in ((wk_sb, kT), (wq_sb, qT)):
                            ps = psA.tile([128, 512], F32, tag="work", name="ps_kq")
                            for kc in range(4):
                                nc.tensor.matmul(
                                    ps[:],
                                    w_sb[:, kc * 128 : (kc + 1) * 128],
                                    xslc(kc, sl),
                                    start=(kc == 0),
                                    stop=(kc == 3),
                                )
                            # ScalarE drains kT/qT: keeps the DVE queue clear
                            # for the v copies and uses Act's prologue slack
                            nc.scalar.activation(dst[s][:], ps[:], CPY)
                        # S+exp for this slice's 4 key tiles first: ScalarE
                        # streams exps while the PE still produces v below.
                        pts = [
                            attn_S(0, kt, psS, ptp)
                            for kt in range(4 * s, 4 * s + 4)
                        ]
                        # v natural: per 128-token block, accumulate over kc
                        vn = psA.tile([128, 512], F32, tag="work", name="vn")
                        for j in range(4):
                            tb = slice((4 * s + j) * 128, (4 * s + j + 1) * 128)
                            for kc in range(4):
                                nc.tensor.matmul(
                                    vn[:, j * 128 : (j + 1) * 128],
                                    xslc(kc, tb),
                                    wv_sb[:, kc * 128 : (kc + 1) * 128],
                                    start=(kc == 0),
                                    stop=(kc == 3),
                                )
                        for j in range(4):
                            t = 4 * s + j
                            nc.vector.tensor_copy(
                                v[t][:, 0:64], vn[:, j * 128 : j * 128 + 64]
                            )
                            nc.vector.tensor_copy(
                                v[t][:, 65:129], vn[:, j * 128 + 64 : (j + 1) * 128]
                            )
                            nc.gpsimd.memset(v[t][:, 64:65], 1.0)
                            nc.gpsimd.memset(v[t][:, 129:130], 1.0)
                        for i, kt in enumerate(range(4 * s, 4 * s + 4)):
                            attn_PV(kt, pts[i], pv0, pv1)
                        # pre-bake chunk 1's first steps into prologue slack:
                        # ScalarE picks up extra exps where it would idle; the
                        # parked pt tiles feed chunk 1's deferred-PV phase.
                        if s in (2, 3) and n_qc > 1:
                            for kt in (2 * (s - 2), 2 * (s - 2) + 1):
                                prebaked[(1, kt)] = attn_S(1, kt, psS, ptp)

                # ---- steady state: chunks 1..n_qc-1. Chunk qc-1's epilogue is
                # interleaved into chunk qc's kt loop: the first DEFER steps
                # emit only S+exp while the DVE drains the previous chunk's PV
                # accumulators; their PV matmuls are emitted after the drain so
                # the in-order PE never blocks.
                DEFER = 8
                # bc + pj share one 2-buffer pool (same tag): 2 PSUM banks
                # total, and consecutive projections ping-pong between banks.
                with tc.tile_pool(name="psX", bufs=2, space="PSUM") as psX:
                    psB = psP = psX
                    prev_pv = (pv0, pv1)
                    prev_qc = 0
                    outT = None
                    for qc in range(1, n_qc):
                        pv0 = psV.tile([65, 512], F32, tag="pv0", name="pv0")
                        pv1 = psV.tile([65, 512], F32, tag="pv1", name="pv1")
                        # drain FIRST: its DVE/ACT copies must not queue
                        # behind the deferred steps' exp instructions
                        drained = epi_drain(prev_pv[0], prev_pv[1], smp, otp)
                        pts = [
                            prebaked.pop((qc, kt), None)
                            or attn_S(qc, kt, psS, ptp)
                            for kt in range(DEFER)
                        ]
                        for kt in range(DEFER):
                            attn_PV(kt, pts[kt], pv0, pv1)
                        for kt in range(DEFER, n_xt):
                            pt = attn_S(qc, kt, psS, ptp)
                            attn_PV(kt, pt, pv0, pv1)
                            if kt == min(DEFER + 1, 9):
                                outT = epi_scale(*drained, smp, otp, psB)
                            if kt in (9, 12, 15, 18):
                                proj_qtile(prev_qc, (kt - 9) // 3, outT, psP, obp)
                        prev_pv = (pv0, pv1)
                        prev_qc = qc
                    drained = epi_drain(prev_pv[0], prev_pv[1], smp, otp, final=True)
                    outT = epi_scale(*drained, smp, otp, psB)
                    for qs in range(4):
                        proj_qtile(prev_qc, qs, outT, psP, obp, final=True)
    nc.compile()
    return nc


_CACHE = {}


def _get_nc(tokens=N):
    if tokens not in _CACHE:
        _CACHE[tokens] = build(tokens)
    return _CACHE[tokens]


def _prep_w(w_slice):
    """[512, 128] -> [128, 512] fp16 with w_[p, kc*128 + j] = w[kc*128 + p, j]."""
    w = np.asarray(w_slice, dtype=np.float32)
    return np.ascontiguousarray(
        w.reshape(4, 128, 128).transpose(1, 0, 2).reshape(128, 512).astype(np.float16)
    )


def _shard_inputs(x, w_qkv, w_proj):
    in_maps = []
    for c in range(8):
        b, hp = divmod(c, 4)
        o = 128 * hp
        wall = np.concatenate(
            [
                _prep_w(w_qkv[:, o : o + 128]),
                _prep_w(w_qkv[:, 512 + o : 512 + o + 128]),
                _prep_w(w_qkv[:, 1024 + o : 1024 + o + 128]),
                w_proj[o : o + 128, :].astype(np.float16),
            ],
            axis=1,
        )
        in_maps.append(
            {
                "xt": np.ascontiguousarray(x[b].T.astype(np.float16)),
                "wall": np.ascontiguousarray(wall),
            }
        )
    return in_maps


def run(x, w_qkv, w_proj, b_proj, trace=False, **kwargs):
    from concourse.bass_utils import run_bass_kernel_spmd

    nc = _get_nc()
    in_maps = _shard_inputs(np.asarray(x), np.asarray(w_qkv), np.asarray(w_proj))
    br = run_bass_kernel_spmd(nc, in_maps, list(range(8)), trace=trace, **kwargs)
    parts = [np.asarray(br.results[c]["out"]) for c in range(8)]
    bp = np.asarray(b_proj)
    o0 = parts[0] + parts[1] + parts[2] + parts[3] + bp
    o1 = parts[4] + parts[5] + parts[6] + parts[7] + bp
    return np.stack([o0, o1]).astype(np.float32), br


def kernel(x, w_qkv, w_proj, b_proj):
    result, _ = run(x, w_qkv, w_proj, b_proj, trace=False)
    return result



# revision 29
# speedup vs baseline: 1.2159x; 1.0590x over previous
"""Trainium2 Bass kernel for multi-head self-attention (B=2, N=4096, C=512, H=8).

Sharding: 8 cores = 2 batches x 4 head-pairs. Core c handles batch c//4 and
heads {2*(c%4), 2*(c%4)+1}. Each core computes its two heads' attention over
all 4096 tokens and a partial output projection restricted to its heads' 128
channels; the host sums the 4 partials per batch (the tensor-parallel proj
all-reduce) and adds b_proj.

Dataflow (fp16 operands, fp32 PSUM accumulation, scores never in DRAM):
  xT arrives host-pre-transposed [512, 4096] fp16 -> SBUF (no PE transposes)
  kT/qT = (w^T @ xT)              [128, 4096]  (rows 0-63 head0, 64-127 head1)
  v natural = xT-block^T @ wv     [4096, 130]  per tile: [Vh0 | 1 | Vh1 | 1]
  per 512-query chunk, per 128-key tile:
    S^T = kslc^T @ qT   (two row-packed K=64 matmuls -> PSUM [128, 1024])
    P^T = exp(SCALE * S^T) fp16   (ScalarE, straight out of PSUM; a fraction
                                   of steps use a custom DVE exp2 instead to
                                   share the exp load with the vector engine)
    PV += [V|1]^T @ P^T           (PSUM accumulate; row 64 = denominators)
  chunk epilogue: raw PV drain (frees PSUM fast, hidden under deferred S/exp
    of the next chunk), then recip/broadcast/scale off the critical path and
    ONE packed proj matmul per 128-query tile (both heads in one K=128).
"""

import os
import sys

if "/opt/trn_rl_repo" not in sys.path:
    sys.path.insert(0, "/opt/trn_rl_repo")

import numpy as np

import concourse.mybir as mybir
import concourse.tile as tile
from concourse import bacc

B, N, C, H = 2, 4096, 512, 8
D = C // H
SCALE = D**-0.5
F32 = mybir.dt.float32
F16 = mybir.dt.float16
I16 = mybir.dt.int16
MUL = mybir.AluOpType.mult
EXP = mybir.ActivationFunctionType.Exp

MM_DT_NAME = "f16"  # informational (test.py prints it)

# Every step's exp is head-split across engines: ScalarE Exp computes head0's
# P^T tile, the DVE computes head1's via a custom Schraudolph exp2 op (int16
# writeback). Separate destination tiles keep the writers independent.

_EXP_OP = None


def _get_exp_op():
    """Register (once) a custom DVE op computing fp16 exp bit patterns.

    Schraudolph in fp16: bits = relu(x*C0 + C1) with C0 = SCALE*log2e*1024,
    C1 = 15360 - 1024*sigma (sigma = 0.0579 makes the sawtooth mean-zero).
    Rounded to int16 at writeback, the result IS the fp16 encoding of
    ~e^(x*SCALE) (max rel err ~4%, zero-mean; the softmax ratio washes it
    out). relu clamps the x << 0 tail to +0.0.
    """
    global _EXP_OP
    if _EXP_OP is not None:
        return _EXP_OP
    from concourse import dve_ops
    from concourse.dve_spec import Spec, Src0, C0, C1, relu, lower
    from concourse.dve_uop import DveOpSpec

    name = "EXP2F16_ANT"
    spec = Spec(
        body=relu(Src0 * C0 + C1),
        # CoreSim reference (HW rounds the float result at int16 writeback)
        reference=lambda in0, in1, s0, s1, imm2: np.rint(
            np.maximum(in0.astype(np.float32) * s0 + s1, 0.0)
        ),
    )
    existing = [op for op in dve_ops.OPS if op.name == name]
    if existing:
        _EXP_OP = existing[0]
        return _EXP_OP
    opcode = dve_ops._CUSTOM_DVE_ROW_BASE + len(dve_ops.OPS)
    uops = lower(spec, ver="v3")
    ds = DveOpSpec(name=name, opcode=opcode, uops=uops, rd1_en=False)
    sha = ds.sha("v3")
    op = dve_ops.DveOp(name, spec, subdim=False, uops_sha={"v3": sha})
    dve_ops.OPS.append(op)
    dve_ops.CUSTOM_DVE_SPECS[name] = spec
    dve_ops._SUB_OPCODE_FOR_NAME[name] = opcode
    _EXP_OP = op
    return op


def _exp_consts():
    """(C0, C1) for the custom exp op."""
    import math

    c0 = SCALE * math.log2(math.e) * 1024.0
    c1 = 15360.0 - 1024.0 * 0.0579
    return c0, c1


def build(tokens=N):
    T = tokens
    n_xt = T // 128  # key tiles
    n_s = T // 512  # token slices for kT/qT production
    n_qc = T // 512  # query chunks

    exp_op = _get_exp_op()
    ec0, ec1 = _exp_consts()

    nc = bacc.Bacc(None)
    xt = nc.dram_tensor("xt", [C, T], F16, kind="ExternalInput")  # x[b].T
    out = nc.dram_tensor("out", [T, C], F32, kind="ExternalOutput")
    # concatenated [wq | wk | wv | wp], each [128, 512] host-prepped fp16
    # (wq/wk/wv: w_[p, kc*128 + j] = w[kc*128 + p, j]; wp natural rows)
    wall = nc.dram_tensor("wall", [128, 2048], F16, kind="ExternalInput")

    with tile.TileContext(nc) as tc:
        with tc.tile_pool(name="persist", bufs=1) as pp:
            w_all = pp.tile([128, 2048], F16, tag="w_all", name="w_all")
            nc.sync.dma_start(out=w_all[:], in_=wall[:, :])
            wq_sb = w_all[:, 0:512]
            wk_sb = w_all[:, 512:1024]
            wv_sb = w_all[:, 1024:1536]
            wp_sb = w_all[:, 1536:2048]
            # ones row for broadcasting recip rows across 64 partitions
            ones1 = pp.tile([1, 64], F16, tag="ones1")
            nc.gpsimd.memset(ones1[:], 1.0)
            # warm the Exp activation table (~1.3us) during the input DMAs
            # instead of on the first real exp
            dum = pp.tile([1, 1], F32, tag="dum")
            nc.gpsimd.memset(dum[:], 0.0)
            nc.scalar.activation(dum[:], dum[:], EXP, scale=1.0)

            # all 4 c-chunks of xT in one flat tile: chunk kc at cols [kc*T..)
            xall = pp.tile([128, 4 * T], F16, tag="xall", name="xall")

            def xslc(kc, sl):
                return xall[:, kc * T + sl.start : kc * T + sl.stop]
            kT = [
                pp.tile([128, 512], F16, tag=f"kT{s}", name=f"kT{s}")
                for s in range(n_s)
            ]
            qT = [
                pp.tile([128, 512], F16, tag=f"qT{s}", name=f"qT{s}")
                for s in range(n_s)
            ]
            v = [
                pp.tile([128, 130], F16, tag=f"v{t}", name=f"v{t}")
                for t in range(n_xt)
            ]

            def attn_S(qc, kt, psS):
                """Scores + column-split exp for one (chunk, key-tile)."""
                sc = psS.tile([128, 1024], F32, tag="sc", name="sc")
                kslc = kT[kt // 4][:, (kt % 4) * 128 : (kt % 4 + 1) * 128]
                nc.tensor.matmul(
                    sc[:, 0:512],
                    kslc[0:64, :],
                    qT[qc][0:64, :],
                    start=True,
                    stop=True,
                    tile_position=(0, 0),
                )
                nc.tensor.matmul(
                    sc[:, 512:1024],
                    kslc[64:128, :],
                    qT[qc][64:128, :],
                    start=True,
                    stop=True,
                    tile_position=(64, 0),
                )
                pta = ptpa.tile([128, 512], F16, tag="pta", name="pta")
                ptb = ptpb.tile([128, 512], F16, tag="ptb", name="ptb")
                nc.scalar.activation(pta[:], sc[:, 0:512], EXP, scale=SCALE)
                nc.vector._custom_dve(
                    exp_op,
                    out=ptb[:].bitcast(I16),
                    in0=sc[:, 512:1024],
                    s0=ec0,
                    s1=ec1,
                )
                return pta, ptb

            def attn_PV(kt, pt, pv0, pv1, start=None, stop=None):
                pta, ptb = pt
                start = (kt == 0) if start is None else start
                stop = (kt == n_xt - 1) if stop is None else stop
                nc.tensor.matmul(
                    pv0[:], v[kt][:, 0:65], pta[:], start=start, stop=stop
                )
                nc.tensor.matmul(
                    pv1[:], v[kt][:, 65:130], ptb[:], start=start, stop=stop
                )

            CPY = mybir.ActivationFunctionType.Copy

            def epi_drain(pv0, pv1, smp, otp, final=False):
                """PV-bank release: den-row copies (ACT+DVE, separate tiles)
                plus raw PV copies. In the final epilogue ScalarE is idle (no
                more exps) so the praw halves go there."""
                dna = smp.tile([1, 512], F32, tag="dna", name="dna")
                dnb = smp.tile([1, 512], F32, tag="dnb", name="dnb")
                nc.scalar.activation(dna[:], pv0[64:65, :], CPY)
                nc.vector.tensor_copy(dnb[:], pv1[64:65, :])
                praw = otp.tile([128, 512], F32, tag="praw", name="praw")
                if final:
                    nc.scalar.activation(praw[0:64, :], pv0[0:64, :], CPY)
                    nc.scalar.activation(praw[64:128, :], pv1[0:64, :], CPY)
                else:
                    nc.vector.tensor_copy(praw[0:64, :], pv0[0:64, :])
                    nc.vector.tensor_copy(praw[64:128, :], pv1[0:64, :])
                return dna, dnb, praw

            def epi_scale(dna, dnb, praw, smp, otp, psB):
                """Off-critical-path: recip (DVE), fp16 casts (ACT), broadcast
                matmuls, fused scale."""
                rca = smp.tile([1, 512], F32, tag="rca", name="rca")
                rcb = smp.tile([1, 512], F32, tag="rcb", name="rcb")
                nc.vector.reciprocal_approx_fast(rca[:], dna[:])
                nc.vector.reciprocal_approx_fast(rcb[:], dnb[:])
                rha = smp.tile([1, 512], F16, tag="rha", name="rha")
                rhb = smp.tile([1, 512], F16, tag="rhb", name="rhb")
                nc.scalar.activation(rha[:], rca[:], CPY)
                nc.scalar.activation(rhb[:], rcb[:], CPY)
                bc = psB.tile([128, 512], F32, tag="pb", name="bc")
                nc.tensor.matmul(bc[0:64, :], ones1[:], rha[:], start=True, stop=True)
                nc.tensor.matmul(
                    bc[64:128, :],
                    ones1[:],
                    rhb[:],
                    start=True,
                    stop=True,
                    tile_position=(0, 64),
                )
                outT = otp.tile([128, 512], F16, tag="outT", name="outT")
                nc.vector.tensor_tensor(outT[:], praw[:], bc[:], MUL)
                return outT

            def proj_qtile(qc, qs, outT, psP, obp, final=False):
                i = qc * 4 + qs
                pj = psP.tile([128, 512], F32, tag="pb", name="pj")
                nc.tensor.matmul(
                    pj[:],
                    outT[:, qs * 128 : (qs + 1) * 128],
                    wp_sb[:],
                    start=True,
                    stop=True,
                )
                ob = obp.tile([128, 512], F32, tag="ob", name="ob")
                if qs % 2 == 1:
                    nc.scalar.activation(ob[:], pj[:], CPY)
                else:
                    nc.vector.tensor_copy(ob[:], pj[:])
                nc.sync.dma_start(out=out[i * 128 : (i + 1) * 128, :], in_=ob[:])

            with tc.tile_pool(name="ptpa", bufs=6) as ptpa, tc.tile_pool(
                name="ptpb", bufs=6
            ) as ptpb, tc.tile_pool(name="smp", bufs=2) as smp, tc.tile_pool(
                name="otp", bufs=2
            ) as otp, tc.tile_pool(
                name="obp", bufs=2
            ) as obp, tc.tile_pool(
                name="psS", bufs=2, space="PSUM"
            ) as psS, tc.tile_pool(name="psV", bufs=1, space="PSUM") as psV:
                pv0 = psV.tile([65, 512], F32, tag="pv0", name="pv0")
                pv1 = psV.tile([65, 512], F32, tag="pv1", name="pv1")

                # ---- prologue: per 512-token slice produce kT/qT/v, with
                # qc=0's attention interleaved so ScalarE starts early
                with tc.tile_pool(name="psA", bufs=2, space="PSUM") as psA:
                    for s in range(n_s):
                        sl = slice(s * 512, (s + 1) * 512)
                        # one DMA fetches this slice's tokens for all 4 chunks
                        nc.sync.dma_start(
                            out=xall[:].rearrange("p (k t) -> p k t", k=4)[:, :, sl],
                            in_=xt.rearrange("(k p) t -> p k t", k=4)[:, :, sl],
                        )
                        for w_sb, dst in ((wk_sb, kT), (wq_sb, qT)):
                            ps = psA.tile([128, 512], F32, tag="work", name="ps_kq")
                            for kc in range(4):
                                nc.tensor.matmul(
                                    ps[:],
                                    w_sb[:, kc * 128 : (kc + 1) * 128],
                                    xslc(kc, sl),
                                    start=(kc == 0),
                                    stop=(kc == 3),
                                )
                            # ScalarE drains kT/qT: keeps the DVE queue clear
                            # for the v copies and uses Act's prologue slack
                            nc.scalar.activation(dst[s][:], ps[:], CPY)
                        # S+exp for this slice's 4 key tiles first: ScalarE
                        # streams exps while the PE still produces v below.
                        pts = [
                            attn_S(0, kt, psS)
                            for kt in range(4 * s, 4 * s + 4)
                        ]
                        # v natural: per 128-token block, accumulate over kc
                        vn = psA.tile([128, 512], F32, tag="work", name="vn")
                        for j in range(4):
                            tb = slice((4 * s + j) * 128, (4 * s + j + 1) * 128)
                            for kc in range(4):
                                nc.tensor.matmul(
                                    vn[:, j * 128 : (j + 1) * 128],
                                    xslc(kc, tb),
                                    wv_sb[:, kc * 128 : (kc + 1) * 128],
                                    start=(kc == 0),
                                    stop=(kc == 3),
                                )
                        for j in range(4):
                            t = 4 * s + j
                            nc.vector.tensor_copy(
                                v[t][:, 0:64], vn[:, j * 128 : j * 128 + 64]
                            )
                            nc.vector.tensor_copy(
                                v[t][:, 65:129], vn[:, j * 128 + 64 : (j + 1) * 128]
                            )
                            nc.gpsimd.memset(v[t][:, 64:65], 1.0)
                            nc.gpsimd.memset(v[t][:, 129:130], 1.0)
                        for i, kt in enumerate(range(4 * s, 4 * s + 4)):
                            attn_PV(kt, pts[i], pv0, pv1)

                # ---- steady state: chunks 1..n_qc-1, software-pipelined. The
                # flattened loop emits S(g)+exp(g) at global step g and the
                # PV matmuls of step g-PDEPTH right after, so by the time the
                # in-order PE reaches a PV its exp finished ~2 steps ago and
                # the matmul stream never stalls on the exp engines. Chunk
                # epilogues are injected at fixed offsets of the PV stream.
                PDEPTH = 2
                if n_xt >= 20:
                    scale_kt, pj_kts = 6, {9: 0, 12: 1, 15: 2, 18: 3}
                else:
                    scale_kt, pj_kts = 2, {4: 0, 5: 1, 6: 2, 7: 3}
                # bc + pj share one 2-buffer pool (same tag): 2 PSUM banks
                # total, and consecutive projections ping-pong between banks.
                with tc.tile_pool(name="psX", bufs=2, space="PSUM") as psX:
                    psB = psP = psX
                    prev_pv = (pv0, pv1)
                    drained = outT = None
                    inflight = {}
                    total = (n_qc - 1) * n_xt
                    for g in range(total + PDEPTH):
                        if g < total:
                            qc = 1 + g // n_xt
                            inflight[g] = attn_S(qc, g % n_xt, psS)
                        e = g - PDEPTH
                        if e < 0:
                            continue
                        eqc = 1 + e // n_xt
                        ekt = e % n_xt
                        if ekt == 0:
                            # previous chunk's epilogue head: free its pv banks
                            drained = epi_drain(prev_pv[0], prev_pv[1], smp, otp)
                            pv0 = psV.tile([65, 512], F32, tag="pv0", name="pv0")
                            pv1 = psV.tile([65, 512], F32, tag="pv1", name="pv1")
                        attn_PV(ekt, inflight.pop(e), pv0, pv1)
                        if ekt == scale_kt:
                            outT = epi_scale(*drained, smp, otp, psB)
                        elif ekt in pj_kts:
                            proj_qtile(eqc - 1, pj_kts[ekt], outT, psP, obp)
                        if ekt == n_xt - 1:
                            prev_pv = (pv0, pv1)
                    drained = epi_drain(prev_pv[0], prev_pv[1], smp, otp, final=True)
                    outT = epi_scale(*drained, smp, otp, psB)
                    for qs in range(4):
                        proj_qtile(n_qc - 1, qs, outT, psP, obp, final=True)
    nc.compile()
    return nc


_CACHE = {}


def _get_nc(tokens=N):
    if tokens not in _CACHE:
        _CACHE[tokens] = build(tokens)
    return _CACHE[tokens]


def _prep_w(w_slice):
    """[512, 128] -> [128, 512] fp16 with w_[p, kc*128 + j] = w[kc*128 + p, j]."""
    w = np.asarray(w_slice, dtype=np.float32)
    return np.ascontiguousarray(
        w.reshape(4, 128, 128).transpose(1, 0, 2).reshape(128, 512).astype(np.float16)
    )


def _shard_inputs(x, w_qkv, w_proj):
    in_maps = []
    for c in range(8):
        b, hp = divmod(c, 4)
        o = 128 * hp
        wall = np.concatenate(
            [
                _prep_w(w_qkv[:, o : o + 128]),
                _prep_w(w_qkv[:, 512 + o : 512 + o + 128]),
                _prep_w(w_qkv[:, 1024 + o : 1024 + o + 128]),
                w_proj[o : o + 128, :].astype(np.float16),
            ],
            axis=1,
        )
        in_maps.append(
            {
                "xt": np.ascontiguousarray(x[b].T.astype(np.float16)),
                "wall": np.ascontiguousarray(wall),
            }
        )
    return in_maps


def run(x, w_qkv, w_proj, b_proj, trace=False, **kwargs):
    from concourse.bass_utils import run_bass_kernel_spmd

    nc = _get_nc()
    in_maps = _shard_inputs(np.asarray(x), np.asarray(w_qkv), np.asarray(w_proj))
    br = run_bass_kernel_spmd(nc, in_maps, list(range(8)), trace=trace, **kwargs)
    parts = [np.asarray(br.results[c]["out"]) for c in range(8)]
    bp = np.asarray(b_proj)
    o0 = parts[0] + parts[1] + parts[2] + parts[3] + bp
    o1 = parts[4] + parts[5] + parts[6] + parts[7] + bp
    return np.stack([o0, o1]).astype(np.float32), br


def kernel(x, w_qkv, w_proj, b_proj):
    result, _ = run(x, w_qkv, w_proj, b_proj, trace=False)
    return result



# revision 31
# speedup vs baseline: 1.3730x; 1.1292x over previous
"""Trainium2 Bass kernel for multi-head self-attention (B=2, N=4096, C=512, H=8).

Sharding: 8 cores = 2 batches x 4 head-pairs. Core c handles batch c//4 and
heads {2*(c%4), 2*(c%4)+1}. Each core computes its two heads' attention over
all 4096 tokens and a partial output projection restricted to its heads' 128
channels; the host sums the 4 partials per batch (the tensor-parallel proj
all-reduce) and adds b_proj.

Dataflow (fp16 operands, fp32 PSUM accumulation, scores never in DRAM):
  xT arrives host-pre-transposed [512, 4096] fp16 -> SBUF (no PE transposes)
  kT/qT = (w^T @ xT)              [128, 4096]  (rows 0-63 head0, 64-127 head1)
  v natural = xT-block^T @ wv     [4096, 130]  per tile: [Vh0 | 1 | Vh1 | 1]
  per 512-query chunk, per 128-key tile:
    S^T = kslc^T @ qT   (two row-packed K=64 matmuls -> PSUM [128, 1024])
    P^T = exp(SCALE * S^T) fp16   (ScalarE, straight out of PSUM; a fraction
                                   of steps use a custom DVE exp2 instead to
                                   share the exp load with the vector engine)
    PV += [V|1]^T @ P^T           (PSUM accumulate; row 64 = denominators)
  chunk epilogue: raw PV drain (frees PSUM fast, hidden under deferred S/exp
    of the next chunk), then recip/broadcast/scale off the critical path and
    ONE packed proj matmul per 128-query tile (both heads in one K=128).
"""

import os
import sys

if "/opt/trn_rl_repo" not in sys.path:
    sys.path.insert(0, "/opt/trn_rl_repo")

import numpy as np

import concourse.mybir as mybir
import concourse.tile as tile
from concourse import bacc

B, N, C, H = 2, 4096, 512, 8
D = C // H
SCALE = D**-0.5
F32 = mybir.dt.float32
F16 = mybir.dt.float16
I16 = mybir.dt.int16
MUL = mybir.AluOpType.mult
EXP = mybir.ActivationFunctionType.Exp

MM_DT_NAME = "f16"  # informational (test.py prints it)

# Every step's exp is head-split across engines: ScalarE Exp computes head0's
# P^T tile, the DVE computes head1's via a custom Schraudolph exp2 op (int16
# writeback). Separate destination tiles keep the writers independent.

_EXP_OP = None


def _get_exp_op():
    """Register (once) a custom DVE op computing fp16 exp bit patterns.

    Schraudolph in fp16: bits = relu(x*C0 + C1) with C0 = SCALE*log2e*1024,
    C1 = 15360 - 1024*sigma (sigma = 0.0579 makes the sawtooth mean-zero).
    Rounded to int16 at writeback, the result IS the fp16 encoding of
    ~e^(x*SCALE) (max rel err ~4%, zero-mean; the softmax ratio washes it
    out). relu clamps the x << 0 tail to +0.0.
    """
    global _EXP_OP
    if _EXP_OP is not None:
        return _EXP_OP
    from concourse import dve_ops
    from concourse.dve_spec import Spec, Src0, C0, C1, relu, lower
    from concourse.dve_uop import DveOpSpec

    name = "EXP2F16_ANT"
    spec = Spec(
        body=relu(Src0 * C0 + C1),
        # CoreSim reference (HW rounds the float result at int16 writeback)
        reference=lambda in0, in1, s0, s1, imm2: np.rint(
            np.maximum(in0.astype(np.float32) * s0 + s1, 0.0)
        ),
    )
    existing = [op for op in dve_ops.OPS if op.name == name]
    if existing:
        _EXP_OP = existing[0]
        return _EXP_OP
    opcode = dve_ops._CUSTOM_DVE_ROW_BASE + len(dve_ops.OPS)
    uops = lower(spec, ver="v3")
    ds = DveOpSpec(name=name, opcode=opcode, uops=uops, rd1_en=False)
    sha = ds.sha("v3")
    op = dve_ops.DveOp(name, spec, subdim=False, uops_sha={"v3": sha})
    dve_ops.OPS.append(op)
    dve_ops.CUSTOM_DVE_SPECS[name] = spec
    dve_ops._SUB_OPCODE_FOR_NAME[name] = opcode
    _EXP_OP = op
    return op


def _exp_consts():
    """(C0, C1) for the custom exp op."""
    import math

    c0 = SCALE * math.log2(math.e) * 1024.0
    c1 = 15360.0 - 1024.0 * 0.0579
    return c0, c1


def build(tokens=N):
    T = tokens
    n_xt = T // 128  # key tiles
    n_s = T // 512  # token slices for kT/qT production
    n_qc = T // 512  # query chunks

    exp_op = _get_exp_op()
    ec0, ec1 = _exp_consts()

    nc = bacc.Bacc(None)
    xt = nc.dram_tensor("xt", [C, T], F16, kind="ExternalInput")  # x[b].T
    out = nc.dram_tensor("out", [T, C], F32, kind="ExternalOutput")
    # concatenated [wq | wk | wv | wp], each [128, 512] host-prepped fp16
    # (wq/wk/wv: w_[p, kc*128 + j] = w[kc*128 + p, j]; wp natural rows)
    wall = nc.dram_tensor("wall", [128, 2048], F16, kind="ExternalInput")

    with tile.TileContext(nc) as tc:
        with tc.tile_pool(name="persist", bufs=1) as pp:
            w_all = pp.tile([128, 2048], F16, tag="w_all", name="w_all")
            nc.sync.dma_start(out=w_all[:], in_=wall[:, :])
            wq_sb = w_all[:, 0:512]
            wk_sb = w_all[:, 512:1024]
            wv_sb = w_all[:, 1024:1536]
            wp_sb = w_all[:, 1536:2048]
            # ones row for broadcasting recip rows across 64 partitions
            ones1 = pp.tile([1, 64], F16, tag="ones1")
            nc.gpsimd.memset(ones1[:], 1.0)
            # warm the Exp activation table (~1.3us) during the input DMAs
            # instead of on the first real exp
            dum = pp.tile([1, 1], F32, tag="dum")
            nc.gpsimd.memset(dum[:], 0.0)
            nc.scalar.activation(dum[:], dum[:], EXP, scale=1.0)

            # all 4 c-chunks of xT in one flat tile: chunk kc at cols [kc*T..)
            xall = pp.tile([128, 4 * T], F16, tag="xall", name="xall")

            def xslc(kc, sl):
                return xall[:, kc * T + sl.start : kc * T + sl.stop]
            kT = [
                pp.tile([128, 512], F16, tag=f"kT{s}", name=f"kT{s}")
                for s in range(n_s)
            ]
            qT = [
                pp.tile([128, 512], F16, tag=f"qT{s}", name=f"qT{s}")
                for s in range(n_s)
            ]
            v = [
                pp.tile([128, 130], F16, tag=f"v{t}", name=f"v{t}")
                for t in range(n_xt)
            ]

            def attn_S(qc, kt):
                """Scores + head-split exp for one (chunk, key-tile). Each
                head gets its own PSUM bank and exp engine so the two exp
                pipelines never gate each other."""
                sca = psSa.tile([128, 512], F32, tag="sca", name="sca")
                scb = psSb.tile([128, 512], F32, tag="scb", name="scb")
                kslc = kT[kt // 4][:, (kt % 4) * 128 : (kt % 4 + 1) * 128]
                nc.tensor.matmul(
                    sca[:],
                    kslc[0:64, :],
                    qT[qc][0:64, :],
                    start=True,
                    stop=True,
                    tile_position=(0, 0),
                )
                nc.tensor.matmul(
                    scb[:],
                    kslc[64:128, :],
                    qT[qc][64:128, :],
                    start=True,
                    stop=True,
                    tile_position=(64, 0),
                )
                pta = ptpa.tile([128, 512], F16, tag="pta", name="pta")
                ptb = ptpb.tile([128, 512], F16, tag="ptb", name="ptb")
                nc.scalar.activation(pta[:], sca[:], EXP, scale=SCALE)
                nc.vector._custom_dve(
                    exp_op,
                    out=ptb[:].bitcast(I16),
                    in0=scb[:],
                    s0=ec0,
                    s1=ec1,
                )
                return pta, ptb

            def attn_PV(kt, pt, pv0, pv1, start=None, stop=None):
                pta, ptb = pt
                start = (kt == 0) if start is None else start
                stop = (kt == n_xt - 1) if stop is None else stop
                nc.tensor.matmul(
                    pv0[:], v[kt][:, 0:65], pta[:], start=start, stop=stop
                )
                nc.tensor.matmul(
                    pv1[:], v[kt][:, 65:130], ptb[:], start=start, stop=stop
                )

            CPY = mybir.ActivationFunctionType.Copy

            def epi_drain(pv0, pv1, smp, otp, final=False):
                """PV-bank release: den-row copies (ACT+DVE, separate tiles)
                plus raw PV copies. In the final epilogue ScalarE is idle (no
                more exps) so the praw halves go there."""
                dna = smp.tile([1, 512], F32, tag="dna", name="dna")
                dnb = smp.tile([1, 512], F32, tag="dnb", name="dnb")
                nc.scalar.activation(dna[:], pv0[64:65, :], CPY)
                nc.vector.tensor_copy(dnb[:], pv1[64:65, :])
                praw = otp.tile([128, 512], F32, tag="praw", name="praw")
                nc.scalar.activation(praw[0:64, :], pv0[0:64, :], CPY)
                if final:
                    nc.scalar.activation(praw[64:128, :], pv1[0:64, :], CPY)
                else:
                    nc.vector.tensor_copy(praw[64:128, :], pv1[0:64, :])
                return dna, dnb, praw

            def epi_scale(dna, dnb, praw, smp, otp, psB):
                """Off-critical-path: recip (DVE), fp16 casts (ACT), broadcast
                matmuls, fused scale."""
                rca = smp.tile([1, 512], F32, tag="rca", name="rca")
                rcb = smp.tile([1, 512], F32, tag="rcb", name="rcb")
                nc.vector.reciprocal_approx_fast(rca[:], dna[:])
                nc.vector.reciprocal_approx_fast(rcb[:], dnb[:])
                rha = smp.tile([1, 512], F16, tag="rha", name="rha")
                rhb = smp.tile([1, 512], F16, tag="rhb", name="rhb")
                nc.vector.tensor_copy(rha[:], rca[:])
                nc.vector.tensor_copy(rhb[:], rcb[:])
                bc = psB.tile([128, 512], F32, tag="pb", name="bc")
                nc.tensor.matmul(bc[0:64, :], ones1[:], rha[:], start=True, stop=True)
                nc.tensor.matmul(
                    bc[64:128, :],
                    ones1[:],
                    rhb[:],
                    start=True,
                    stop=True,
                    tile_position=(0, 64),
                )
                outT = otp.tile([128, 512], F16, tag="outT", name="outT")
                nc.vector.tensor_tensor(outT[:], praw[:], bc[:], MUL)
                return outT

            def proj_qtile(qc, qs, outT, psP, obp, final=False):
                i = qc * 4 + qs
                pj = psP.tile([128, 512], F32, tag="pb", name="pj")
                nc.tensor.matmul(
                    pj[:],
                    outT[:, qs * 128 : (qs + 1) * 128],
                    wp_sb[:],
                    start=True,
                    stop=True,
                )
                ob = obp.tile([128, 512], F32, tag="ob", name="ob")
                nc.scalar.activation(ob[:], pj[:], CPY)
                nc.sync.dma_start(out=out[i * 128 : (i + 1) * 128, :], in_=ob[:])

            with tc.tile_pool(name="ptpa", bufs=6) as ptpa, tc.tile_pool(
                name="ptpb", bufs=6
            ) as ptpb, tc.tile_pool(name="smp", bufs=2) as smp, tc.tile_pool(
                name="otp", bufs=2
            ) as otp, tc.tile_pool(
                name="obp", bufs=2
            ) as obp, tc.tile_pool(
                name="psSa", bufs=2, space="PSUM"
            ) as psSa, tc.tile_pool(
                name="psSb", bufs=2, space="PSUM"
            ) as psSb, tc.tile_pool(name="psV", bufs=1, space="PSUM") as psV:
                pv0 = psV.tile([65, 512], F32, tag="pv0", name="pv0")
                pv1 = psV.tile([65, 512], F32, tag="pv1", name="pv1")

                # ---- prologue: per 512-token slice produce kT/qT/v, with
                # qc=0's attention interleaved so ScalarE starts early
                with tc.tile_pool(name="psA", bufs=2, space="PSUM") as psA:
                    for s in range(n_s):
                        sl = slice(s * 512, (s + 1) * 512)
                        # one DMA fetches this slice's tokens for all 4 chunks
                        nc.sync.dma_start(
                            out=xall[:].rearrange("p (k t) -> p k t", k=4)[:, :, sl],
                            in_=xt.rearrange("(k p) t -> p k t", k=4)[:, :, sl],
                        )
                        for w_sb, dst in ((wk_sb, kT), (wq_sb, qT)):
                            ps = psA.tile([128, 512], F32, tag="work", name="ps_kq")
                            for kc in range(4):
                                nc.tensor.matmul(
                                    ps[:],
                                    w_sb[:, kc * 128 : (kc + 1) * 128],
                                    xslc(kc, sl),
                                    start=(kc == 0),
                                    stop=(kc == 3),
                                )
                            # ScalarE drains kT/qT: keeps the DVE queue clear
                            # for the v copies and uses Act's prologue slack
                            nc.scalar.activation(dst[s][:], ps[:], CPY)
                        # S+exp for this slice's 4 key tiles first: ScalarE
                        # streams exps while the PE still produces v below.
                        pts = [
                            attn_S(0, kt)
                            for kt in range(4 * s, 4 * s + 4)
                        ]
                        # v natural: per 128-token block, accumulate over kc
                        vn = psA.tile([128, 512], F32, tag="work", name="vn")
                        for j in range(4):
                            tb = slice((4 * s + j) * 128, (4 * s + j + 1) * 128)
                            for kc in range(4):
                                nc.tensor.matmul(
                                    vn[:, j * 128 : (j + 1) * 128],
                                    xslc(kc, tb),
                                    wv_sb[:, kc * 128 : (kc + 1) * 128],
                                    start=(kc == 0),
                                    stop=(kc == 3),
                                )
                        for j in range(4):
                            t = 4 * s + j
                            nc.vector.tensor_copy(
                                v[t][:, 0:64], vn[:, j * 128 : j * 128 + 64]
                            )
                            nc.vector.tensor_copy(
                                v[t][:, 65:129], vn[:, j * 128 + 64 : (j + 1) * 128]
                            )
                            nc.gpsimd.memset(v[t][:, 64:65], 1.0)
                            nc.gpsimd.memset(v[t][:, 129:130], 1.0)
                        for i, kt in enumerate(range(4 * s, 4 * s + 4)):
                            attn_PV(kt, pts[i], pv0, pv1)

                # ---- steady state: chunks 1..n_qc-1, software-pipelined. The
                # flattened loop emits S(g)+exp(g) at global step g and the
                # PV matmuls of step g-PDEPTH right after, so by the time the
                # in-order PE reaches a PV its exp finished ~2 steps ago and
                # the matmul stream never stalls on the exp engines. Chunk
                # epilogues are injected at fixed offsets of the PV stream.
                PDEPTH = 4
                if n_xt >= 20:
                    scale_kt, pj_kts = 6, {9: 0, 12: 1, 15: 2, 18: 3}
                else:
                    scale_kt, pj_kts = 2, {4: 0, 5: 1, 6: 2, 7: 3}
                # bc + pj share one 2-buffer pool (same tag): 2 PSUM banks
                # total, and consecutive projections ping-pong between banks.
                with tc.tile_pool(name="psX", bufs=2, space="PSUM") as psX:
                    psB = psP = psX
                    prev_pv = (pv0, pv1)
                    drained = outT = None
                    inflight = {}
                    total = (n_qc - 1) * n_xt
                    for g in range(total + PDEPTH):
                        if g < total:
                            qc = 1 + g // n_xt
                            inflight[g] = attn_S(qc, g % n_xt)
                        e = g - PDEPTH
                        if e < 0:
                            continue
                        eqc = 1 + e // n_xt
                        ekt = e % n_xt
                        if ekt == 0:
                            # previous chunk's epilogue head: free its pv banks
                            drained = epi_drain(prev_pv[0], prev_pv[1], smp, otp)
                            pv0 = psV.tile([65, 512], F32, tag="pv0", name="pv0")
                            pv1 = psV.tile([65, 512], F32, tag="pv1", name="pv1")
                        attn_PV(ekt, inflight.pop(e), pv0, pv1)
                        if ekt == scale_kt:
                            outT = epi_scale(*drained, smp, otp, psB)
                        elif ekt in pj_kts:
                            proj_qtile(eqc - 1, pj_kts[ekt], outT, psP, obp)
                        if ekt == n_xt - 1:
                            prev_pv = (pv0, pv1)
                    drained = epi_drain(prev_pv[0], prev_pv[1], smp, otp, final=True)
                    outT = epi_scale(*drained, smp, otp, psB)
                    for qs in range(4):
                        proj_qtile(n_qc - 1, qs, outT, psP, obp, final=True)
    nc.compile()
    return nc


_CACHE = {}


def _get_nc(tokens=N):
    if tokens not in _CACHE:
        _CACHE[tokens] = build(tokens)
    return _CACHE[tokens]


def _prep_w(w_slice):
    """[512, 128] -> [128, 512] fp16 with w_[p, kc*128 + j] = w[kc*128 + p, j]."""
    w = np.asarray(w_slice, dtype=np.float32)
    return np.ascontiguousarray(
        w.reshape(4, 128, 128).transpose(1, 0, 2).reshape(128, 512).astype(np.float16)
    )


def _shard_inputs(x, w_qkv, w_proj):
    in_maps = []
    for c in range(8):
        b, hp = divmod(c, 4)
        o = 128 * hp
        wall = np.concatenate(
            [
                _prep_w(w_qkv[:, o : o + 128]),
                _prep_w(w_qkv[:, 512 + o : 512 + o + 128]),
                _prep_w(w_qkv[:, 1024 + o : 1024 + o + 128]),
                w_proj[o : o + 128, :].astype(np.float16),
            ],
            axis=1,
        )
        in_maps.append(
            {
                "xt": np.ascontiguousarray(x[b].T.astype(np.float16)),
                "wall": np.ascontiguousarray(wall),
            }
        )
    return in_maps


def run(x, w_qkv, w_proj, b_proj, trace=False, **kwargs):
    from concourse.bass_utils import run_bass_kernel_spmd

    nc = _get_nc()
    in_maps = _shard_inputs(np.asarray(x), np.asarray(w_qkv), np.asarray(w_proj))
    br = run_bass_kernel_spmd(nc, in_maps, list(range(8)), trace=trace, **kwargs)
    parts = [np.asarray(br.results[c]["out"]) for c in range(8)]
    bp = np.asarray(b_proj)
    o0 = parts[0] + parts[1] + parts[2] + parts[3] + bp
    o1 = parts[4] + parts[5] + parts[6] + parts[7] + bp
    return np.stack([o0, o1]).astype(np.float32), br


def kernel(x, w_qkv, w_proj, b_proj):
    result, _ = run(x, w_qkv, w_proj, b_proj, trace=False)
    return result

